# revision 7
# baseline (speedup 1.0000x reference)
"""MoE FFN (8 experts, top-2, SwiGLU) Trainium2 kernel — bf16, ld-amortized.

Expert-parallel: core e holds expert e's weights. The router (top-2
selection + combine weights) runs on host, exactly replicating the
reference; tokens are dispatched to the cores owning their top-2 experts
and the host scatter-adds the per-expert partial outputs back.

FFN matmuls run in bf16 (measured on HW: same per-moving-row rate as
fp8 DoubleRow, ~0.21 ns/row, so plain bf16 beats compensated fp8).
Loops are weight-stationary (j-outer): each 128-row weight tile is
loaded into the PE array once and swept across the full 1152-token
moving dim (LDWEIGHTS costs ~73 ns serial per reload, so reloads per
chunk would add ~35%).

Self-contained: shapes/sharding hardcoded for
x[2,2048,1024], 8 experts, d_expert=2048, top-2.
"""

import math
from contextlib import ExitStack

import ml_dtypes
import numpy as np

import concourse.bass as bass
import concourse.mybir as mybir
import concourse.tile as tile
from concourse import bacc
from concourse.bass_utils import run_bass_kernel_spmd
from concourse.masks import make_identity

# ---- problem constants --------------------------------------------------
B, T, D = 2, 2048, 1024
N_TOK = B * T          # 4096 tokens
E = 8                  # experts == cores
H = 2048               # expert hidden dim
TOP_K = 2
P = 128

CAP = 1152             # per-expert token capacity per dispatch round
NT = CAP // P          # 9  token tiles
ND = D // P            # 8  d-tiles
NH = H // P            # 16 h-tiles
NWG = 4                # wg/wv DMA column groups

BFD = mybir.dt.bfloat16
FP = mybir.dt.float32
AF = mybir.ActivationFunctionType
OP = mybir.AluOpType
BF = ml_dtypes.bfloat16

CHUNKS = [(0, 512), (512, 512), (1024, 128)]
assert sum(w for _, w in CHUNKS) == CAP


def _emit(nc, tc, ctx, x_d, wg_d, wv_d, wo_d, wc_d, y_d):
    const = ctx.enter_context(tc.tile_pool(name="const", bufs=1))
    wsb = ctx.enter_context(tc.tile_pool(name="wsb", bufs=1))
    htp = ctx.enter_context(tc.tile_pool(name="ht", bufs=1))
    act = ctx.enter_context(tc.tile_pool(name="act", bufs=3))
    yst = ctx.enter_context(tc.tile_pool(name="yst", bufs=3))

    ident = const.tile([P, P], FP)
    make_identity(nc, ident[:])
    wc_sb = const.tile([P, NT], FP)
    nc.scalar.dma_start(out=wc_sb[:], in_=wc_d.ap())

    x_sb = const.tile([P, ND, CAP], BFD)
    x_ap = x_d.ap().rearrange("(j p) c -> p j c", p=P)
    wg_sb = wsb.tile([P, ND, H], BFD)
    wv_sb = wsb.tile([P, ND, H], BFD)
    wg_ap = wg_d.ap().rearrange("(j p) h -> p j h", p=P)
    wv_ap = wv_d.ap().rearrange("(j p) h -> p j h", p=P)
    wo_sb = wsb.tile([P, NH, D], BFD)
    wo_ap = wo_d.ap().rearrange("(j p) d -> p j d", p=P)

    GW = H // NWG  # 512 h-columns per wg/wv DMA group

    def g_sl(g):
        return slice(g * GW, (g + 1) * GW)

    # head-latency-ordered loads: phase A h-tile 0 needs x[j=0..7] + group 0
    nc.sync.dma_start(out=x_sb[:, 0:4], in_=x_ap[:, 0:4])
    nc.sync.dma_start(out=wg_sb[:, :, g_sl(0)], in_=wg_ap[:, :, g_sl(0)])
    nc.sync.dma_start(out=x_sb[:, 4:8], in_=x_ap[:, 4:8])
    nc.sync.dma_start(out=wv_sb[:, :, g_sl(0)], in_=wv_ap[:, :, g_sl(0)])
    for g in range(1, NWG):
        nc.sync.dma_start(out=wg_sb[:, :, g_sl(g)], in_=wg_ap[:, :, g_sl(g)])
        nc.sync.dma_start(out=wv_sb[:, :, g_sl(g)], in_=wv_ap[:, :, g_sl(g)])
    nc.sync.dma_start(out=wo_sb[:, 0:8], in_=wo_ap[:, 0:8])
    nc.sync.dma_start(out=wo_sb[:, 8:16], in_=wo_ap[:, 8:16])

    ht = [htp.tile([P, CAP], BFD, name=f"ht{k}") for k in range(NH)]

    # PE p-state warm-up while the head DMAs land
    with ExitStack() as wctx:
        ps_w = wctx.enter_context(tc.tile_pool(name="psw", bufs=1, space="PSUM"))
        warm = ps_w.tile([P, P], FP, name="warm", tag="warm")
        for _ in range(10):
            nc.tensor.matmul(warm[:], lhsT=ident[:], rhs=ident[:],
                             start=True, stop=True)

    # ---- phase A: hT[h, tok] = silu(x@wg)^T * (x@wv)^T ------------------
    # g-pass then v-pass share psum tags (generational cycling): sigmoid
    # and t1 run mid-tile on the g results, freeing banks before the next
    # h-tile's matmuls need them.
    with ExitStack() as actx:
        ps_a = actx.enter_context(tc.tile_pool(name="psa", bufs=3, space="PSUM"))
        for hk in range(NH):
            hs = slice(hk * P, (hk + 1) * P)
            pgs = [ps_a.tile([P, cw], FP, name=f"pg{ci}", tag=f"p{ci}",
                             bufs=(2 if cw == 128 else 3))
                   for ci, (_, cw) in enumerate(CHUNKS)]
            for j in range(ND):
                lhsT = wg_sb[:, j, hs]
                for ci, (cs, cw) in enumerate(CHUNKS):
                    nc.tensor.matmul(
                        pgs[ci][:], lhsT=lhsT,
                        rhs=x_sb[:, j, cs:cs + cw],
                        start=(j == 0), stop=(j == ND - 1))
            pvs = [ps_a.tile([P, cw], FP, name=f"pv{ci}", tag=f"p{ci}",
                             bufs=(2 if cw == 128 else 3))
                   for ci, (_, cw) in enumerate(CHUNKS)]
            for j in range(ND):
                lhsT = wv_sb[:, j, hs]
                for ci, (cs, cw) in enumerate(CHUNKS):
                    nc.tensor.matmul(
                        pvs[ci][:], lhsT=lhsT,
                        rhs=x_sb[:, j, cs:cs + cw],
                        start=(j == 0), stop=(j == ND - 1))
            for ci, (cs, cw) in enumerate(CHUNKS):
                sg = act.tile([P, 512], FP, tag="sg")
                nc.scalar.activation(sg[:, :cw], pgs[ci][:], AF.Sigmoid)
                t1 = act.tile([P, 512], FP, tag="t1")
                nc.vector.tensor_tensor(t1[:, :cw], pgs[ci][:], sg[:, :cw],
                                        op=OP.mult)
                nc.vector.tensor_tensor(ht[hk][:, cs:cs + cw], t1[:, :cw],
                                        pvs[ci][:], op=OP.mult)

    # ---- phase B: y[tok, d] = (hT^T @ wo) * combine ---------------------
    with ExitStack() as bctx:
        ps_y = bctx.enter_context(tc.tile_pool(name="psy", bufs=3, space="PSUM"))
        for tt in range(NT):
            ts = slice(tt * P, (tt + 1) * P)
            pys = [ps_y.tile([P, 512], FP, name=f"py{dc}", tag=f"py{dc}")
                   for dc in range(2)]
            for j in range(NH):
                lhsT = ht[j][:, ts]
                for dc in range(2):
                    nc.tensor.matmul(
                        pys[dc][:], lhsT=lhsT,
                        rhs=wo_sb[:, j, dc * 512:(dc + 1) * 512],
                        start=(j == 0), stop=(j == NH - 1))
            ysb = yst.tile([P, D], FP, tag="y")
            for dc in range(2):
                nc.scalar.activation(ysb[:, dc * 512:(dc + 1) * 512],
                                     pys[dc][:], AF.Copy,
                                     scale=wc_sb[:, tt:tt + 1])
            nc.gpsimd.dma_start(out=y_d.ap()[ts, :], in_=ysb[:])


def _dedup_ldweights(nc):
    """Drop InstLdweights that reload the exact weights already resident
    in the PE array (identical AP/mode, no intervening clobber, no sync).
    The PE weight registers persist across matmuls, so the following
    non-self-loading matmuls keep using the loaded values; each removed
    reload saves ~73 ns of serial PE time."""
    removed = 0
    for blk in nc.main_func.blocks:
        last_key = None
        new = []
        for inst in blk.instructions:
            if isinstance(inst, mybir.InstLdweights):
                si = inst.sync_info
                clean = si is None or (not si.on_wait and not si.on_update)
                key = (
                    repr(inst.ins[0]), str(inst.perf_mode),
                    str(inst.is_transpose), str(inst.tile_position),
                    str(inst.tile_size),
                )
                if clean and key == last_key:
                    removed += 1
                    continue
                last_key = key
            elif isinstance(inst, mybir.InstMatmult):
                # self-loading (fp32) or transpose matmuls clobber the array
                if inst.ldweights is None or inst.is_transpose:
                    last_key = None
            new.append(inst)
        blk.instructions[:] = new
    return removed


def _build():
    nc = bacc.Bacc("TRN2", target_bir_lowering=False, debug=False)
    x_d = nc.dram_tensor("x", [D, CAP], BFD, kind="ExternalInput")
    wg_d = nc.dram_tensor("wg", [D, H], BFD, kind="ExternalInput")
    wv_d = nc.dram_tensor("wv", [D, H], BFD, kind="ExternalInput")
    wo_d = nc.dram_tensor("wo", [H, D], BFD, kind="ExternalInput")
    wc_d = nc.dram_tensor("wc", [P, NT], FP, kind="ExternalInput")
    y_d = nc.dram_tensor("y", [CAP, D], FP, kind="ExternalOutput")
    with tile.TileContext(nc) as tc:
        with ExitStack() as ctx:
            _emit(nc, tc, ctx, x_d, wg_d, wv_d, wo_d, wc_d, y_d)
    _dedup_ldweights(nc)
    nc.compile()
    return nc


_NC = None


def _get_nc():
    global _NC
    if _NC is None:
        _NC = _build()
    return _NC


def _route(xf, gate_w, expert_bias):
    """Host-side replica of the reference router."""
    logits = xf @ gate_w + expert_bias          # [N, E] fp32
    m = logits.max(axis=-1, keepdims=True)
    p = np.exp(logits - m)
    p /= p.sum(axis=-1, keepdims=True)
    # ties -> lower index first, matching jax.lax.top_k
    order = np.argsort(-p, axis=-1, kind="stable")[:, :TOP_K]
    rw = np.take_along_axis(p, order, -1)
    rw = rw / (rw.sum(-1, keepdims=True) + np.float32(1e-8))
    return order, rw


def kernel(x, gate_w, expert_bias, w_gate, w_value, w_out, _trace=False):
    x = np.asarray(x, dtype=np.float32)
    gate_w = np.asarray(gate_w, dtype=np.float32)
    expert_bias = np.asarray(expert_bias, dtype=np.float32)
    w_gate = np.asarray(w_gate, dtype=np.float32)
    w_value = np.asarray(w_value, dtype=np.float32)
    w_out = np.asarray(w_out, dtype=np.float32)

    xf = np.ascontiguousarray(x.reshape(N_TOK, D))
    order, rw = _route(xf, gate_w, expert_bias)
    idx = [np.flatnonzero((order == e).any(axis=-1)) for e in range(E)]
    n_rounds = max(1, math.ceil(max(len(i) for i in idx) / CAP))

    nc = _get_nc()
    wg_b = [np.ascontiguousarray(w_gate[e].astype(BF)) for e in range(E)]
    wv_b = [np.ascontiguousarray(w_value[e].astype(BF)) for e in range(E)]
    wo_b = [np.ascontiguousarray(w_out[e].astype(BF)) for e in range(E)]

    out = np.zeros((N_TOK, D), dtype=np.float32)
    last = None
    for r in range(n_rounds):
        in_maps = []
        for e in range(E):
            ids = idx[e][r * CAP:(r + 1) * CAP]
            ids_p = np.zeros(CAP, dtype=np.int64)
            ids_p[: len(ids)] = ids
            xt = np.ascontiguousarray(xf[ids_p].T.astype(BF))
            sel = order[ids_p] == e                 # [CAP, 2]
            w_tok = np.where(sel[:, 0], rw[ids_p, 0], rw[ids_p, 1])
            wc = np.ascontiguousarray(
                w_tok.astype(np.float32).reshape(NT, P).T)
            in_maps.append({
                "x": xt, "wg": wg_b[e], "wv": wv_b[e], "wo": wo_b[e],
                "wc": wc,
            })
        res = run_bass_kernel_spmd(
            nc, in_maps, core_ids=list(range(E)),
            trace=bool(_trace), trace_cores=list(range(E)) if _trace else None,
        )
        last = res
        for e in range(E):
            ids = idx[e][r * CAP:(r + 1) * CAP]
            if len(ids):
                out[ids] += res.results[e]["y"][: len(ids)]
    if _trace:
        kernel.last_results = last
    return out.reshape(B, T, D)


# revision 11
# speedup vs baseline: 1.0362x; 1.0362x over previous
"""MoE FFN (8 experts, top-2, SwiGLU) Trainium2 kernel — bf16, ld-amortized.

Expert-parallel: core e holds expert e's weights. The router (top-2
selection + combine weights) runs on host, exactly replicating the
reference; tokens are dispatched to the cores owning their top-2 experts
and the host scatter-adds the per-expert partial outputs back.

FFN matmuls run in bf16 (measured on HW: same per-moving-row rate as
fp8 DoubleRow, ~0.21 ns/row, so plain bf16 beats compensated fp8).
Loops are weight-stationary (j-outer): each 128-row weight tile is
loaded into the PE array once and swept across the full 1152-token
moving dim (LDWEIGHTS costs ~73 ns serial per reload, so reloads per
chunk would add ~35%).

Self-contained: shapes/sharding hardcoded for
x[2,2048,1024], 8 experts, d_expert=2048, top-2.
"""

import math
from contextlib import ExitStack

import ml_dtypes
import numpy as np

import concourse.bass as bass
import concourse.mybir as mybir
import concourse.tile as tile
from concourse import bacc
from concourse.bass_utils import run_bass_kernel_spmd

# ---- problem constants --------------------------------------------------
B, T, D = 2, 2048, 1024
N_TOK = B * T          # 4096 tokens
E = 8                  # experts == cores
H = 2048               # expert hidden dim
TOP_K = 2
P = 128

CAP = 1152             # per-expert token capacity per dispatch round
NT = CAP // P          # 9  token tiles
ND = D // P            # 8  d-tiles
NH = H // P            # 16 h-tiles
NWG = 4                # wg/wv DMA column groups

BFD = mybir.dt.bfloat16
FP = mybir.dt.float32
AF = mybir.ActivationFunctionType
OP = mybir.AluOpType
BF = ml_dtypes.bfloat16

CHUNKS = [(0, 512), (512, 512), (1024, 128)]
assert sum(w for _, w in CHUNKS) == CAP

_IDENT = np.ascontiguousarray(np.eye(P, dtype=np.float32).astype(BF))


def _emit(nc, tc, ctx, x_d, wg_d, wv_d, wo_d, wc_d, id_d, y_d):
    const = ctx.enter_context(tc.tile_pool(name="const", bufs=1))
    wsb = ctx.enter_context(tc.tile_pool(name="wsb", bufs=1))
    htp = ctx.enter_context(tc.tile_pool(name="ht", bufs=1))
    act = ctx.enter_context(tc.tile_pool(name="act", bufs=3))
    yst = ctx.enter_context(tc.tile_pool(name="yst", bufs=3))

    # host-supplied bf16 identity: warm-up needs no gpsimd iota, starts
    # as soon as this ~32KB DMA lands on the scalar queue (~0.5us)
    ident = const.tile([P, P], BFD)
    nc.scalar.dma_start(out=ident[:], in_=id_d.ap())
    wc_sb = const.tile([P, NT], FP)
    nc.scalar.dma_start(out=wc_sb[:], in_=wc_d.ap())

    x_sb = const.tile([P, ND, CAP], BFD)
    x_ap = x_d.ap().rearrange("(j p) c -> p j c", p=P)
    wg_sb = wsb.tile([P, ND, H], BFD)
    wv_sb = wsb.tile([P, ND, H], BFD)
    wg_ap = wg_d.ap().rearrange("(j p) h -> p j h", p=P)
    wv_ap = wv_d.ap().rearrange("(j p) h -> p j h", p=P)
    wo_sb = wsb.tile([P, NH, D], BFD)
    wo_ap = wo_d.ap().rearrange("(j p) d -> p j d", p=P)

    GW = H // NWG  # 512 h-columns per wg/wv DMA group

    def g_sl(g):
        return slice(g * GW, (g + 1) * GW)

    # head-latency-ordered loads: h-tile 0's g-pass needs x[j] + wg cols
    # 0:128 first; its v-pass needs wv cols 0:128 a few us later.
    nc.sync.dma_start(out=x_sb[:, 0:2], in_=x_ap[:, 0:2])
    nc.sync.dma_start(out=wg_sb[:, :, 0:P], in_=wg_ap[:, :, 0:P])
    nc.sync.dma_start(out=x_sb[:, 2:4], in_=x_ap[:, 2:4])
    nc.sync.dma_start(out=wv_sb[:, :, 0:P], in_=wv_ap[:, :, 0:P])
    nc.sync.dma_start(out=x_sb[:, 4:6], in_=x_ap[:, 4:6])
    nc.sync.dma_start(out=x_sb[:, 6:8], in_=x_ap[:, 6:8])
    nc.sync.dma_start(out=wg_sb[:, :, P:GW], in_=wg_ap[:, :, P:GW])
    nc.sync.dma_start(out=wv_sb[:, :, P:GW], in_=wv_ap[:, :, P:GW])
    for g in range(1, NWG):
        nc.sync.dma_start(out=wg_sb[:, :, g_sl(g)], in_=wg_ap[:, :, g_sl(g)])
        nc.sync.dma_start(out=wv_sb[:, :, g_sl(g)], in_=wv_ap[:, :, g_sl(g)])
    nc.sync.dma_start(out=wo_sb[:, 0:8], in_=wo_ap[:, 0:8])
    nc.sync.dma_start(out=wo_sb[:, 8:16], in_=wo_ap[:, 8:16])

    ht = [htp.tile([P, CAP], BFD, name=f"ht{k}") for k in range(NH)]

    # PE p-state warm-up while the head DMAs land (~3.5us of junk)
    with ExitStack() as wctx:
        ps_w = wctx.enter_context(tc.tile_pool(name="psw", bufs=1, space="PSUM"))
        warm = ps_w.tile([P, P], FP, name="warm", tag="warm")
        for _ in range(24):
            nc.tensor.matmul(warm[:], lhsT=ident[:], rhs=ident[:],
                             start=True, stop=True)

    # ---- phase A: hT[h, tok] = silu(x@wg)^T * (x@wv)^T ------------------
    # g-pass then v-pass share psum tags (generational cycling): sigmoid
    # and t1 run mid-tile on the g results, freeing banks before the next
    # h-tile's matmuls need them.
    with ExitStack() as actx:
        ps_a = actx.enter_context(tc.tile_pool(name="psa", bufs=3, space="PSUM"))
        for hk in range(NH):
            hs = slice(hk * P, (hk + 1) * P)
            pgs = [ps_a.tile([P, cw], FP, name=f"pg{ci}", tag=f"p{ci}",
                             bufs=(2 if cw == 128 else 3))
                   for ci, (_, cw) in enumerate(CHUNKS)]
            for j in range(ND):
                lhsT = wg_sb[:, j, hs]
                for ci, (cs, cw) in enumerate(CHUNKS):
                    nc.tensor.matmul(
                        pgs[ci][:], lhsT=lhsT,
                        rhs=x_sb[:, j, cs:cs + cw],
                        start=(j == 0), stop=(j == ND - 1))
            pvs = [ps_a.tile([P, cw], FP, name=f"pv{ci}", tag=f"p{ci}",
                             bufs=(2 if cw == 128 else 3))
                   for ci, (_, cw) in enumerate(CHUNKS)]
            for j in range(ND):
                lhsT = wv_sb[:, j, hs]
                for ci, (cs, cw) in enumerate(CHUNKS):
                    nc.tensor.matmul(
                        pvs[ci][:], lhsT=lhsT,
                        rhs=x_sb[:, j, cs:cs + cw],
                        start=(j == 0), stop=(j == ND - 1))
            for ci, (cs, cw) in enumerate(CHUNKS):
                sg = act.tile([P, 512], FP, tag="sg")
                nc.scalar.activation(sg[:, :cw], pgs[ci][:], AF.Sigmoid)
                t1 = act.tile([P, 512], FP, tag="t1")
                nc.vector.tensor_tensor(t1[:, :cw], pgs[ci][:], sg[:, :cw],
                                        op=OP.mult)
                nc.vector.tensor_tensor(ht[hk][:, cs:cs + cw], t1[:, :cw],
                                        pvs[ci][:], op=OP.mult)

    # ---- phase B: y[tok, d] = (hT^T @ wo) * combine ---------------------
    with ExitStack() as bctx:
        ps_y = bctx.enter_context(tc.tile_pool(name="psy", bufs=3, space="PSUM"))
        for tt in range(NT):
            ts = slice(tt * P, (tt + 1) * P)
            pys = [ps_y.tile([P, 512], FP, name=f"py{dc}", tag=f"py{dc}")
                   for dc in range(2)]
            for j in range(NH):
                lhsT = ht[j][:, ts]
                for dc in range(2):
                    nc.tensor.matmul(
                        pys[dc][:], lhsT=lhsT,
                        rhs=wo_sb[:, j, dc * 512:(dc + 1) * 512],
                        start=(j == 0), stop=(j == NH - 1))
            ysb = yst.tile([P, D], FP, tag="y")
            for dc in range(2):
                nc.scalar.activation(ysb[:, dc * 512:(dc + 1) * 512],
                                     pys[dc][:], AF.Copy,
                                     scale=wc_sb[:, tt:tt + 1])
            nc.gpsimd.dma_start(out=y_d.ap()[ts, :], in_=ysb[:])


def _dedup_ldweights(nc):
    """Drop InstLdweights that reload the exact weights already resident
    in the PE array (identical AP/mode, no intervening clobber, no sync).
    The PE weight registers persist across matmuls, so the following
    non-self-loading matmuls keep using the loaded values; each removed
    reload saves ~73 ns of serial PE time."""
    removed = 0
    for blk in nc.main_func.blocks:
        last_key = None
        new = []
        for inst in blk.instructions:
            if isinstance(inst, mybir.InstLdweights):
                si = inst.sync_info
                clean = si is None or (not si.on_wait and not si.on_update)
                key = (
                    repr(inst.ins[0]), str(inst.perf_mode),
                    str(inst.is_transpose), str(inst.tile_position),
                    str(inst.tile_size),
                )
                if clean and key == last_key:
                    removed += 1
                    continue
                last_key = key
            elif isinstance(inst, mybir.InstMatmult):
                # self-loading (fp32) or transpose matmuls clobber the array
                if inst.ldweights is None or inst.is_transpose:
                    last_key = None
            new.append(inst)
        blk.instructions[:] = new
    return removed


def _build():
    nc = bacc.Bacc("TRN2", target_bir_lowering=False, debug=False)
    x_d = nc.dram_tensor("x", [D, CAP], BFD, kind="ExternalInput")
    wg_d = nc.dram_tensor("wg", [D, H], BFD, kind="ExternalInput")
    wv_d = nc.dram_tensor("wv", [D, H], BFD, kind="ExternalInput")
    wo_d = nc.dram_tensor("wo", [H, D], BFD, kind="ExternalInput")
    wc_d = nc.dram_tensor("wc", [P, NT], FP, kind="ExternalInput")
    id_d = nc.dram_tensor("id", [P, P], BFD, kind="ExternalInput")
    y_d = nc.dram_tensor("y", [CAP, D], FP, kind="ExternalOutput")
    with tile.TileContext(nc) as tc:
        with ExitStack() as ctx:
            _emit(nc, tc, ctx, x_d, wg_d, wv_d, wo_d, wc_d, id_d, y_d)
    _dedup_ldweights(nc)
    nc.compile()
    return nc


_NC = None


def _get_nc():
    global _NC
    if _NC is None:
        _NC = _build()
    return _NC


def _route(xf, gate_w, expert_bias):
    """Host-side replica of the reference router."""
    logits = xf @ gate_w + expert_bias          # [N, E] fp32
    m = logits.max(axis=-1, keepdims=True)
    p = np.exp(logits - m)
    p /= p.sum(axis=-1, keepdims=True)
    # ties -> lower index first, matching jax.lax.top_k
    order = np.argsort(-p, axis=-1, kind="stable")[:, :TOP_K]
    rw = np.take_along_axis(p, order, -1)
    rw = rw / (rw.sum(-1, keepdims=True) + np.float32(1e-8))
    return order, rw


def kernel(x, gate_w, expert_bias, w_gate, w_value, w_out, _trace=False):
    x = np.asarray(x, dtype=np.float32)
    gate_w = np.asarray(gate_w, dtype=np.float32)
    expert_bias = np.asarray(expert_bias, dtype=np.float32)
    w_gate = np.asarray(w_gate, dtype=np.float32)
    w_value = np.asarray(w_value, dtype=np.float32)
    w_out = np.asarray(w_out, dtype=np.float32)

    xf = np.ascontiguousarray(x.reshape(N_TOK, D))
    order, rw = _route(xf, gate_w, expert_bias)
    idx = [np.flatnonzero((order == e).any(axis=-1)) for e in range(E)]
    n_rounds = max(1, math.ceil(max(len(i) for i in idx) / CAP))

    nc = _get_nc()
    wg_b = [np.ascontiguousarray(w_gate[e].astype(BF)) for e in range(E)]
    wv_b = [np.ascontiguousarray(w_value[e].astype(BF)) for e in range(E)]
    wo_b = [np.ascontiguousarray(w_out[e].astype(BF)) for e in range(E)]

    out = np.zeros((N_TOK, D), dtype=np.float32)
    last = None
    for r in range(n_rounds):
        in_maps = []
        for e in range(E):
            ids = idx[e][r * CAP:(r + 1) * CAP]
            ids_p = np.zeros(CAP, dtype=np.int64)
            ids_p[: len(ids)] = ids
            xt = np.ascontiguousarray(xf[ids_p].T.astype(BF))
            sel = order[ids_p] == e                 # [CAP, 2]
            w_tok = np.where(sel[:, 0], rw[ids_p, 0], rw[ids_p, 1])
            wc = np.ascontiguousarray(
                w_tok.astype(np.float32).reshape(NT, P).T)
            in_maps.append({
                "x": xt, "wg": wg_b[e], "wv": wv_b[e], "wo": wo_b[e],
                "wc": wc, "id": _IDENT,
            })
        res = run_bass_kernel_spmd(
            nc, in_maps, core_ids=list(range(E)),
            trace=bool(_trace), trace_cores=list(range(E)) if _trace else None,
        )
        last = res
        for e in range(E):
            ids = idx[e][r * CAP:(r + 1) * CAP]
            if len(ids):
                out[ids] += res.results[e]["y"][: len(ids)]
    if _trace:
        kernel.last_results = last
    return out.reshape(B, T, D)


# revision 12
# speedup vs baseline: 1.0835x; 1.0457x over previous
"""MoE FFN (8 experts, top-2, SwiGLU) Trainium2 kernel — bf16 half-expert.

Sharding: each expert's hidden dim H=2048 is split into two halves; the
16 (expert, H-half) jobs are packed 2-per-core. Slot 0 holds halves of
the 4 heaviest-loaded experts (per the host router) with token capacity
1092, slot 1 the 4 lightest with capacity 1024 — vs 1152 for plain
expert-parallel SPMD, a ~8% cut in padded matmul rows. The two halves
of an expert produce partial y sums the host adds during scatter.

The router (top-2 + combine weights) runs on host, exactly replicating
the reference. FFN matmuls are bf16 at the measured PE rate of 1 cycle
per moving row with weight loads fully hidden; loops are structured so
each 128-row weight tile sweeps all resident tokens.

Self-contained: shapes/sharding hardcoded for
x[2,2048,1024], 8 experts, d_expert=2048, top-2.
"""

import math
from contextlib import ExitStack

import ml_dtypes
import numpy as np

import concourse.bass as bass
import concourse.mybir as mybir
import concourse.tile as tile
from concourse import bacc
from concourse.bass_utils import run_bass_kernel_spmd

# ---- problem constants --------------------------------------------------
B, T, D = 2, 2048, 1024
N_TOK = B * T          # 4096 tokens
E = 8                  # experts
H = 2048               # expert hidden dim
HH = H // 2            # per-slot hidden half
TOP_K = 2
P = 128

CAP0 = 1092            # slot-0 token capacity (4 heaviest experts)
CAP1 = 1024            # slot-1 token capacity (4 lightest experts)
ND = D // P            # 8  d-tiles
NHH = HH // P          # 8  h-tiles per slot
NT0 = (CAP0 + P - 1) // P   # 9 (last tile 68 tokens)
NT1 = CAP1 // P             # 8

BFD = mybir.dt.bfloat16
FP = mybir.dt.float32
AF = mybir.ActivationFunctionType
OP = mybir.AluOpType
BF = ml_dtypes.bfloat16

CH0 = [(0, 512), (512, 512), (1024, CAP0 - 1024)]
CH1 = [(0, 512), (512, 512)]

_IDENT = np.ascontiguousarray(np.eye(P, dtype=np.float32).astype(BF))


def _emit(nc, tc, ctx, tens):
    const = ctx.enter_context(tc.tile_pool(name="const", bufs=1))
    wsb = ctx.enter_context(tc.tile_pool(name="wsb", bufs=1))
    htp = ctx.enter_context(tc.tile_pool(name="htp", bufs=1))
    act = ctx.enter_context(tc.tile_pool(name="act", bufs=2))
    yst = ctx.enter_context(tc.tile_pool(name="yst", bufs=2))

    ident = const.tile([P, P], BFD)
    wc0_sb = const.tile([P, NT0], FP)
    nc.scalar.dma_start(out=wc0_sb[:], in_=tens["wc0"].ap())
    wc1_sb = const.tile([P, NT1], FP)
    nc.scalar.dma_start(out=wc1_sb[:], in_=tens["wc1"].ap())

    caps = (CAP0, CAP1)
    chunks = (CH0, CH1)
    nts = (NT0, NT1)
    x_sb, wg_sb, wv_sb, wo_sb, wc_sb = [], [], [], [], [wc0_sb, wc1_sb]
    x_ap, wg_ap, wv_ap, wo_ap = [], [], [], []
    for s in range(2):
        x_sb.append(const.tile([P, ND, caps[s]], BFD, name=f"x{s}"))
        x_ap.append(tens[f"x{s}"].ap().rearrange("(j p) c -> p j c", p=P))
        wg_sb.append(wsb.tile([P, ND, HH], BFD, name=f"wg{s}"))
        wg_ap.append(tens[f"wg{s}"].ap().rearrange("(j p) h -> p j h", p=P))
        wv_sb.append(wsb.tile([P, ND, HH], BFD, name=f"wv{s}"))
        wv_ap.append(tens[f"wv{s}"].ap().rearrange("(j p) h -> p j h", p=P))
        wo_sb.append(wsb.tile([P, NHH, D], BFD, name=f"wo{s}"))
        wo_ap.append(tens[f"wo{s}"].ap().rearrange("(j p) d -> p j d", p=P))

    # head-latency-ordered loads (sync queue): identity first (unblocks
    # PE warm-up), then slot0's x + first weight columns, then the rest.
    nc.sync.dma_start(out=ident[:], in_=tens["id"].ap())
    nc.sync.dma_start(out=x_sb[0][:, 0:2], in_=x_ap[0][:, 0:2])
    nc.sync.dma_start(out=wg_sb[0][:, :, 0:P], in_=wg_ap[0][:, :, 0:P])
    nc.sync.dma_start(out=x_sb[0][:, 2:4], in_=x_ap[0][:, 2:4])
    nc.sync.dma_start(out=wv_sb[0][:, :, 0:P], in_=wv_ap[0][:, :, 0:P])
    nc.sync.dma_start(out=x_sb[0][:, 4:6], in_=x_ap[0][:, 4:6])
    nc.sync.dma_start(out=x_sb[0][:, 6:8], in_=x_ap[0][:, 6:8])
    nc.sync.dma_start(out=wg_sb[0][:, :, P:HH], in_=wg_ap[0][:, :, P:HH])
    nc.sync.dma_start(out=wv_sb[0][:, :, P:HH], in_=wv_ap[0][:, :, P:HH])
    nc.sync.dma_start(out=x_sb[1][:, 0:4], in_=x_ap[1][:, 0:4])
    nc.sync.dma_start(out=x_sb[1][:, 4:8], in_=x_ap[1][:, 4:8])
    nc.sync.dma_start(out=wg_sb[1][:], in_=wg_ap[1])
    nc.sync.dma_start(out=wv_sb[1][:], in_=wv_ap[1])
    nc.sync.dma_start(out=wo_sb[0][:], in_=wo_ap[0])
    nc.sync.dma_start(out=wo_sb[1][:], in_=wo_ap[1])

    ht = [[htp.tile([P, caps[s]], BFD, name=f"ht{s}_{k}") for k in range(NHH)]
          for s in range(2)]

    # PE p-state warm-up while the head DMAs land (~2.5us of junk)
    with ExitStack() as wctx:
        ps_w = wctx.enter_context(tc.tile_pool(name="psw", bufs=1, space="PSUM"))
        warm = ps_w.tile([P, P], FP, name="warm", tag="warm")
        for _ in range(24):
            nc.tensor.matmul(warm[:], lhsT=ident[:], rhs=ident[:],
                             start=True, stop=True)

    # ---- phase A: hT[h, tok] = silu(x@wg)^T * (x@wv)^T ------------------
    # g-pass then v-pass share psum tags (generational cycling): sigmoid
    # and t1 run mid-tile on the g results, freeing banks early.
    with ExitStack() as actx:
        ps_a = actx.enter_context(tc.tile_pool(name="psa", bufs=3, space="PSUM"))
        for s in range(2):
            for hk in range(NHH):
                hs = slice(hk * P, (hk + 1) * P)
                pgs = [ps_a.tile([P, cw], FP, name=f"pg{ci}", tag=f"p{ci}",
                                 bufs=(2 if ci == 2 else 3))
                       for ci, (_, cw) in enumerate(chunks[s])]
                for j in range(ND):
                    lhsT = wg_sb[s][:, j, hs]
                    for ci, (cs, cw) in enumerate(chunks[s]):
                        nc.tensor.matmul(
                            pgs[ci][:], lhsT=lhsT,
                            rhs=x_sb[s][:, j, cs:cs + cw],
                            start=(j == 0), stop=(j == ND - 1))
                pvs = [ps_a.tile([P, cw], FP, name=f"pv{ci}", tag=f"p{ci}",
                                 bufs=(2 if ci == 2 else 3))
                       for ci, (_, cw) in enumerate(chunks[s])]
                for j in range(ND):
                    lhsT = wv_sb[s][:, j, hs]
                    for ci, (cs, cw) in enumerate(chunks[s]):
                        nc.tensor.matmul(
                            pvs[ci][:], lhsT=lhsT,
                            rhs=x_sb[s][:, j, cs:cs + cw],
                            start=(j == 0), stop=(j == ND - 1))
                for ci, (cs, cw) in enumerate(chunks[s]):
                    sg = act.tile([P, 512], FP, tag="sg")
                    nc.scalar.activation(sg[:, :cw], pgs[ci][:], AF.Sigmoid)
                    t1 = act.tile([P, 512], FP, tag="t1")
                    nc.vector.tensor_tensor(t1[:, :cw], pgs[ci][:],
                                            sg[:, :cw], op=OP.mult)
                    nc.vector.tensor_tensor(ht[s][hk][:, cs:cs + cw],
                                            t1[:, :cw], pvs[ci][:],
                                            op=OP.mult)

    # ---- phase B: y[tok, d] = (hT^T @ wo) * combine ---------------------
    with ExitStack() as bctx:
        ps_y = bctx.enter_context(tc.tile_pool(name="psy", bufs=3, space="PSUM"))
        for s in range(2):
            y_ap = tens[f"y{s}"].ap()
            for tt in range(nts[s]):
                tw = min(P, caps[s] - tt * P)
                ts = slice(tt * P, tt * P + tw)
                pys = [ps_y.tile([P, 512], FP, name=f"py{dc}", tag=f"py{dc}")
                       for dc in range(2)]
                for j in range(NHH):
                    lhsT = ht[s][j][:, ts]
                    for dc in range(2):
                        nc.tensor.matmul(
                            pys[dc][:tw, :], lhsT=lhsT,
                            rhs=wo_sb[s][:, j, dc * 512:(dc + 1) * 512],
                            start=(j == 0), stop=(j == NHH - 1))
                ysb = yst.tile([P, D], FP, tag="y")
                for dc in range(2):
                    nc.scalar.activation(ysb[:tw, dc * 512:(dc + 1) * 512],
                                         pys[dc][:tw, :], AF.Copy,
                                         scale=wc_sb[s][:tw, tt:tt + 1])
                nc.gpsimd.dma_start(out=y_ap[ts, :], in_=ysb[:tw, :])


def _dedup_ldweights(nc):
    """Drop InstLdweights that reload the exact weights already resident
    in the PE array (identical AP/mode, no intervening clobber, no sync)."""
    removed = 0
    for blk in nc.main_func.blocks:
        last_key = None
        new = []
        for inst in blk.instructions:
            if isinstance(inst, mybir.InstLdweights):
                si = inst.sync_info
                clean = si is None or (not si.on_wait and not si.on_update)
                key = (
                    repr(inst.ins[0]), str(inst.perf_mode),
                    str(inst.is_transpose), str(inst.tile_position),
                    str(inst.tile_size),
                )
                if clean and key == last_key:
                    removed += 1
                    continue
                last_key = key
            elif isinstance(inst, mybir.InstMatmult):
                if inst.ldweights is None or inst.is_transpose:
                    last_key = None
            new.append(inst)
        blk.instructions[:] = new
    return removed


def _build():
    nc = bacc.Bacc("TRN2", target_bir_lowering=False, debug=False)
    tens = {
        "x0": nc.dram_tensor("x0", [D, CAP0], BFD, kind="ExternalInput"),
        "x1": nc.dram_tensor("x1", [D, CAP1], BFD, kind="ExternalInput"),
        "wg0": nc.dram_tensor("wg0", [D, HH], BFD, kind="ExternalInput"),
        "wg1": nc.dram_tensor("wg1", [D, HH], BFD, kind="ExternalInput"),
        "wv0": nc.dram_tensor("wv0", [D, HH], BFD, kind="ExternalInput"),
        "wv1": nc.dram_tensor("wv1", [D, HH], BFD, kind="ExternalInput"),
        "wo0": nc.dram_tensor("wo0", [HH, D], BFD, kind="ExternalInput"),
        "wo1": nc.dram_tensor("wo1", [HH, D], BFD, kind="ExternalInput"),
        "wc0": nc.dram_tensor("wc0", [P, NT0], FP, kind="ExternalInput"),
        "wc1": nc.dram_tensor("wc1", [P, NT1], FP, kind="ExternalInput"),
        "id": nc.dram_tensor("id", [P, P], BFD, kind="ExternalInput"),
        "y0": nc.dram_tensor("y0", [CAP0, D], FP, kind="ExternalOutput"),
        "y1": nc.dram_tensor("y1", [CAP1, D], FP, kind="ExternalOutput"),
    }
    with tile.TileContext(nc) as tc:
        with ExitStack() as ctx:
            _emit(nc, tc, ctx, tens)
    _dedup_ldweights(nc)
    nc.compile()
    return nc


_NC = None


def _get_nc():
    global _NC
    if _NC is None:
        _NC = _build()
    return _NC


def _route(xf, gate_w, expert_bias):
    """Host-side replica of the reference router."""
    logits = xf @ gate_w + expert_bias          # [N, E] fp32
    m = logits.max(axis=-1, keepdims=True)
    p = np.exp(logits - m)
    p /= p.sum(axis=-1, keepdims=True)
    # ties -> lower index first, matching jax.lax.top_k
    order = np.argsort(-p, axis=-1, kind="stable")[:, :TOP_K]
    rw = np.take_along_axis(p, order, -1)
    rw = rw / (rw.sum(-1, keepdims=True) + np.float32(1e-8))
    return order, rw


def _slot_inputs(xf, order, rw, ids, cap, nt, w_gate, w_value, w_out,
                 expert, half):
    """Build one (expert, H-half) job's device inputs."""
    ids_p = np.zeros(cap, dtype=np.int64)
    ids_p[: len(ids)] = ids
    xt = np.ascontiguousarray(xf[ids_p].T.astype(BF))
    sel = order[ids_p] == expert
    w_tok = np.where(sel[:, 0], rw[ids_p, 0], rw[ids_p, 1]).astype(np.float32)
    w_tok[len(ids):] = 0.0
    wc = np.zeros(nt * P, dtype=np.float32)
    wc[:cap] = w_tok
    wc = np.ascontiguousarray(wc.reshape(nt, P).T)
    hsl = slice(half * HH, (half + 1) * HH)
    return {
        "x": xt,
        "wg": np.ascontiguousarray(w_gate[expert][:, hsl].astype(BF)),
        "wv": np.ascontiguousarray(w_value[expert][:, hsl].astype(BF)),
        "wo": np.ascontiguousarray(w_out[expert][hsl, :].astype(BF)),
        "wc": wc,
    }


def kernel(x, gate_w, expert_bias, w_gate, w_value, w_out, _trace=False):
    x = np.asarray(x, dtype=np.float32)
    gate_w = np.asarray(gate_w, dtype=np.float32)
    expert_bias = np.asarray(expert_bias, dtype=np.float32)
    w_gate = np.asarray(w_gate, dtype=np.float32)
    w_value = np.asarray(w_value, dtype=np.float32)
    w_out = np.asarray(w_out, dtype=np.float32)

    xf = np.ascontiguousarray(x.reshape(N_TOK, D))
    order, rw = _route(xf, gate_w, expert_bias)
    idx = [np.flatnonzero((order == e).any(axis=-1)) for e in range(E)]

    # slot 0 <- 4 heaviest experts, slot 1 <- 4 lightest
    by_load = sorted(range(E), key=lambda e: -len(idx[e]))
    slot_exp = (by_load[:4], by_load[4:])
    caps = (CAP0, CAP1)
    nts = (NT0, NT1)
    n_rounds = max(
        max(1, math.ceil(len(idx[e]) / caps[s]))
        for s in range(2) for e in slot_exp[s]
    )

    nc = _get_nc()
    out = np.zeros((N_TOK, D), dtype=np.float32)
    last = None
    for r in range(n_rounds):
        in_maps = []
        round_ids = [[], []]
        for c in range(E):
            m = {"id": _IDENT}
            for s in range(2):
                e = slot_exp[s][c // 2]
                half = c % 2
                ids = idx[e][r * caps[s]:(r + 1) * caps[s]]
                round_ids[s].append(ids)
                job = _slot_inputs(xf, order, rw, ids, caps[s], nts[s],
                                   w_gate, w_value, w_out, e, half)
                m[f"x{s}"] = job["x"]
                m[f"wg{s}"] = job["wg"]
                m[f"wv{s}"] = job["wv"]
                m[f"wo{s}"] = job["wo"]
                m[f"wc{s}"] = job["wc"]
            in_maps.append(m)
        res = run_bass_kernel_spmd(
            nc, in_maps, core_ids=list(range(E)),
            trace=bool(_trace), trace_cores=list(range(E)) if _trace else None,
        )
        last = res
        for c in range(E):
            for s in range(2):
                ids = round_ids[s][c]
                if len(ids):
                    out[ids] += res.results[c][f"y{s}"][: len(ids)]
    if _trace:
        kernel.last_results = last
    return out.reshape(B, T, D)
